# revision 36
# baseline (speedup 1.0000x reference)
"""Trainium2 Bass kernel for nn_BaoCypherNet (tree-conv GNN).

Data-parallel over 8 NeuronCores: each core processes 256 trees.

v6 design — gathers as one-hot matmuls on the PE (the v1 GPSIMD
ap_gather was the 8.8ms bottleneck; SWDGE dma_gather is descriptor-
capped; DMA xbar transpose measured ~26 GB/s — all rejected).

Key structure:
  - Activations live NODE-MAJOR (X^T[node, channel], fp16) so each
    layer chains without transposes:
      gather:  G[c, m] = sum_n X^T[n, c] * S[n, m]   (X^T stationary,
               host-built one-hot S [n, 3*128] streams; S is shared by
               layers 2 and 3 since the indices repeat)
      conv:    Z^T[m, o] = sum_c G[c, m] * W^T[c, o]  (gathered G chunk
               stationary, replicated weights stream)
    Z^T lands node-major in PSUM, so the ACT Prelu(PSUM->SBUF) writes
    the next layer's X^T directly.
  - Layer 1's gather depends only on the inputs, so the HOST
    pre-gathers trees into G1 [c, tree, 384] fp16; it also precomputes
    the L1 LN mean exactly (nm1D, replicated across partitions).
  - S dummy columns (col 0 of each 128-block) are ZEROED on the host,
    so every conv's node-0 output row is exactly 0 pre-mean; the LN
    mean-subtract is fused into the Prelu bias (per-partition AP), so
    node 0 becomes leaky(-mean) with no fixup ops and no K=1
    mean-broadcast matmuls on the PE.
  - LN scale is deferred through layers 1-2 (leaky(s*x) = s*leaky(x));
    only LN3's variance is computed (Square+reduce), batched with the
    whole stats/MLP tail once per 16-tree supergroup.
  - PSUM: two half-bank z1 tiles rotate (next pair's convs only wait
    on the previous pair's Prelus); one shared 4-slot pool (psX) for
    all gather outputs + small stats matmuls.
  - Gather-output PSUM->SBUF copies are split between ACT and DVE to
    balance the two (both ~73% busy, PE ~75%).
  - Supergroups of 16 trees; phases A/B/C/D software-pipelined one
    supergroup apart; input DMA sliced per 4-tree subgroup so compute
    starts early.
"""

import numpy as np

import bass_rust as _bass_rust
import concourse.bass as bass
import concourse.mybir as mybir
from concourse.bass_utils import run_bass_kernel_spmd
from concourse.tile import TileContext

F32 = mybir.dt.float32
F16 = mybir.dt.float16

N_CORES = 8
B = 2048
BC = B // N_CORES   # trees per core (256)
N = 128             # nodes (incl. zero-pad node 0)
TG = 16             # trees per supergroup
TC = 4              # trees per compute sub-group
NSG = BC // TG      # supergroups per core (16)
SUBS = TG // TC     # sub-groups per supergroup (4)
K1 = 256 * 128      # LN element counts per tree
K2 = 128 * 128
K3 = 64 * 128

_ALPHA = 0.01


def _ap(t_ap, extra_dims, offset_delta=0):
    """AP on the same tensor: keep partition dim, replace free dims."""
    return bass.AP(
        tensor=t_ap.tensor,
        offset=t_ap.offset + offset_delta,
        ap=[t_ap.ap[0]] + list(extra_dims),
    )


def build_nc(with_bias: bool):
    nc = bass.Bass()

    g1_in = nc.dram_tensor("g1D", [128, BC, 384], F16, kind="ExternalInput")
    s_in = nc.dram_tensor("sD", [128, BC, 384], F16, kind="ExternalInput")
    nm1_in = nc.dram_tensor("nm1D", [128, BC], F16, kind="ExternalInput")
    w1_in = nc.dram_tensor("w1tT", [128, 3, 256], F16, kind="ExternalInput")
    w2_in = nc.dram_tensor("w2tT", [128, 2, 3, 128], F16, kind="ExternalInput")
    w3_in = nc.dram_tensor("w3tT", [128, 3, 64], F16, kind="ExternalInput")
    ones_in = nc.dram_tensor("onesrow", [1, 128], F16, kind="ExternalInput")
    ones128_in = nc.dram_tensor("ones128", [128, 128], F16, kind="ExternalInput")
    brow_in = nc.dram_tensor("brows", [3, 256], F16, kind="ExternalInput")
    mlp_in = nc.dram_tensor("mlp_rhs", [66, 32], F32, kind="ExternalInput")
    w5_in = nc.dram_tensor("w5rep", [128, 32], F32, kind="ExternalInput")
    b5_in = nc.dram_tensor("b5rep", [128, 1], F32, kind="ExternalInput")
    out_dram = nc.dram_tensor("out", [BC, 1], F32, kind="ExternalOutput")

    with TileContext(nc) as tc:
        with (
            tc.tile_pool(name="const", bufs=1) as cp,
            tc.tile_pool(name="sx", bufs=2) as sx,
            tc.tile_pool(name="gp", bufs=2) as gp,
            tc.tile_pool(name="psZ1", bufs=2, space="PSUM") as psZ1,
            tc.tile_pool(name="psB", bufs=2, space="PSUM") as psB,
            tc.tile_pool(name="psX", bufs=4, space="PSUM") as psX,
        ):
            # ---- constants ----
            w1tT = cp.tile([128, 3, 256], F16, tag="w1tT")
            nc.sync.dma_start(out=w1tT[:], in_=w1_in[:])
            w2tT = cp.tile([128, 2, 3, 128], F16, tag="w2tT")
            nc.sync.dma_start(out=w2tT[:], in_=w2_in[:])
            w3tT = cp.tile([128, 3, 64], F16, tag="w3tT")
            nc.sync.dma_start(out=w3tT[:], in_=w3_in[:])
            nm1sb = cp.tile([128, BC], F16, tag="nm1sb")
            nc.sync.dma_start(out=nm1sb[:], in_=nm1_in[:])

            live = {}

            def phase_in(i):
                # per-subgroup DMA slices so compute on the first trees can
                # start before the whole supergroup has landed
                g1 = gp.tile([128, TG, 384], F16, tag="g1", bufs=3)
                st = gp.tile([128, TG, 384], F16, tag="st", bufs=5)
                for s in range(SUBS):
                    lo, hi = s * TC, (s + 1) * TC
                    nc.sync.dma_start(
                        out=g1[:, lo:hi, :],
                        in_=g1_in[:, i * TG + lo:i * TG + hi, :])
                    nc.sync.dma_start(
                        out=st[:, lo:hi, :],
                        in_=s_in[:, i * TG + lo:i * TG + hi, :])
                live[i] = {"g1": g1, "s": st}

            def phase_A(sg, s):
                g1 = live[sg]["g1"]
                if s == 0:
                    live[sg]["x2t"] = sx.tile([128, TG, 256], F16, tag="x2t", name="x2t", bufs=3)
                x2t = live[sg]["x2t"]
                if True:
                    t0 = s * TC
                    # two half-tiles (1 PSUM bank each, 2-buf rotation) so
                    # the next pair's convs only wait on this pair's Prelus
                    for h in range(2):
                        z1 = psZ1.tile([128, 2, 256], F32, tag="z1")
                        first = True
                        if with_bias:
                            nc.tensor.matmul(
                                z1[:], onesrow[:],
                                _ap(brows[0:1, :], [[0, 2], [1, 256]]),
                                start=True, stop=False, skip_group_check=True)
                            first = False
                        for u in range(2):
                            t = 2 * h + u
                            for k in range(3):
                                nc.tensor.matmul(
                                    z1[:, u, :],
                                    g1[:, t0 + t, k * 128:(k + 1) * 128],
                                    w1tT[:, k, :],
                                    start=(first and k == 0 and u == 0),
                                    stop=(k == 2 and u == 1),
                                    skip_group_check=True)
                        # mean subtract fused into the Prelu bias (host
                        # precomputed -mean, replicated across partitions).
                        # z1 row 0 = 0 (dummy col) -> leaky(-mean).
                        for u in range(2):
                            t = 2 * h + u
                            nc.scalar.activation(
                                x2t[:, t0 + t, :], z1[:, u, :],
                                mybir.ActivationFunctionType.Prelu,
                                bias=nm1sb[:, sg * TG + t0 + t:
                                           sg * TG + t0 + t + 1],
                                scale=1.0, alpha=_ALPHA)

            def phase_B(sg, s):
                st = live[sg]["s"]
                x2t = live[sg]["x2t"]
                if s == 0:
                    live[sg]["x3t"] = sx.tile([128, TG, 128], F16, tag="x3t", name="x3t", bufs=3)
                x3t = live[sg]["x3t"]
                if True:
                    t0 = s * TC
                    # gather via one-hot matmul: G2[c,m] = X2^T . S
                    z2 = psB.tile([128, TC, 128], F32, tag="zB")
                    first = True
                    if with_bias:
                        nc.tensor.matmul(
                            z2[:], onesrow[:],
                            _ap(brows[1:2, :], [[0, TC], [1, 128]]),
                            start=True, stop=False, skip_group_check=True)
                        first = False
                    # per tree: gather both c-halves, copy, then its convs
                    # immediately -- PE fills copy latency with conv work
                    for t in range(TC):
                        gsb = sx.tile([128, 2, 384], F16, tag=f"g2sb{t % 2}")
                        for j in range(2):
                            gps = psX.tile([128, 384], F32, tag="psX")
                            nc.tensor.matmul(
                                gps[:], x2t[:, t0 + t, j * 128:(j + 1) * 128],
                                st[:, t0 + t, :], start=True, stop=True)
                            with nc.allow_low_precision(reason="fp16 acts"):
                                if j == 1:
                                    nc.vector.tensor_copy(
                                        gsb[:, j, :], gps[:])
                                else:
                                    nc.scalar.activation(
                                        gsb[:, j, :], gps[:],
                                        mybir.ActivationFunctionType.Copy,
                                        bias=0.0, scale=1.0)
                        for k in range(3):
                            for j in range(2):
                                nc.tensor.matmul(
                                    z2[:, t, :],
                                    gsb[:, j, k * 128:(k + 1) * 128],
                                    w2tT[:, j, k, :],
                                    start=(first and k == 0 and j == 0
                                           and t == 0),
                                    stop=(t == TC - 1 and k == 2 and j == 1),
                                    skip_group_check=True)
                    s2 = sx.tile([128, TC], F16, tag="s2")
                    with nc.allow_low_precision(reason="LN mean in fp16 ok"):
                        nc.vector.tensor_reduce(
                            s2[:], z2[:], axis=mybir.AxisListType.X,
                            op=mybir.AluOpType.add)
                    ps2 = psX.tile([128, TC], F32, tag="psX")
                    nc.tensor.matmul(ps2[:], ones128[:], s2[:],
                                     start=True, stop=True)
                    nm2 = sx.tile([128, TC], F16, tag="nm2")
                    with nc.allow_low_precision(reason="LN mean in fp16 ok"):
                        nc.vector.tensor_scalar(
                            nm2[:], ps2[:], -1.0 / K2, None, mybir.AluOpType.mult)
                    # z2 row 0 = 0 by construction (S dummy cols zeroed
                    # on host); mean subtract fused into the Prelu bias.
                    for t in range(TC):
                        nc.scalar.activation(
                            x3t[:, t0 + t, :], z2[:, t, :],
                            mybir.ActivationFunctionType.Prelu,
                            bias=nm2[:, t:t + 1], scale=1.0, alpha=_ALPHA)

            def phase_C(sg, s):
                st = live[sg]["s"]
                x3t = live[sg]["x3t"]
                if True:
                    t0 = s * TC
                    g3sb = sx.tile([128, TC, 384], F16, tag="g3sb")
                    for t in range(TC):
                        gps = psX.tile([128, 384], F32, tag="psX")
                        nc.tensor.matmul(
                            gps[:], x3t[:, t0 + t, :], st[:, t0 + t, :],
                            start=True, stop=True)
                        with nc.allow_low_precision(reason="fp16 acts"):
                            if t % 2 == 1:
                                nc.vector.tensor_copy(
                                    g3sb[:, t, :], gps[:])
                            else:
                                nc.scalar.activation(
                                    g3sb[:, t, :], gps[:],
                                    mybir.ActivationFunctionType.Copy,
                                    bias=0.0, scale=1.0)
                    zb = psB.tile([128, TC, 128], F32, tag="zB")
                    z3 = zb[0:64, :, :]
                    for h in range(2):
                        for k in range(3):
                            rhs = bass.AP(
                                tensor=g3sb.tensor,
                                offset=(g3sb[:].offset + 2 * h * 384
                                        + k * 128),
                                ap=[g3sb[:].ap[0], [384, 2], [1, 128]])
                            nc.tensor.matmul(
                                zb[0:64, 2 * h:2 * h + 2, :],
                                w3tT[:, k, :], rhs,
                                start=(k == 0),
                                stop=(k == 2 and not with_bias),
                                skip_group_check=True)
                    if with_bias:
                        nc.tensor.matmul(
                            z3, brows[2:3, 0:64],
                            _ap(onesrow[:], [[0, TC], [0, 128]]),
                            start=False, stop=True, skip_group_check=True)
                    z3v = zb[0:64, :, 1:128]
                    # LN3 raw stats accumulate into per-supergroup tiles;
                    # the scalar pipeline runs once per supergroup (phase_D)
                    if s == 0:
                        live[sg]["s3g"] = sx.tile([64, 2, TG], F16,
                                                  tag="s3g", name="s3g")
                        live[sg]["prg"] = sx.tile([64, TG], F32,
                                                  tag="prg", name="prg")
                    s3g = live[sg]["s3g"]
                    prg = live[sg]["prg"]
                    sq = sx.tile([64, TC, 127], F16, tag="sq")
                    with nc.allow_low_precision(reason="stats fp16 ok"):
                        nc.vector.tensor_reduce(
                            s3g[:, 0, t0:t0 + TC], z3v,
                            axis=mybir.AxisListType.X, op=mybir.AluOpType.add)
                        nc.scalar.activation(
                            sq[:], z3v, mybir.ActivationFunctionType.Square,
                            bias=0.0, scale=1.0)
                        nc.vector.tensor_reduce(
                            s3g[:, 1, t0:t0 + TC], sq[:],
                            axis=mybir.AxisListType.X, op=mybir.AluOpType.add)
                    nc.vector.tensor_reduce(
                        prg[:, t0:t0 + TC], z3v, axis=mybir.AxisListType.X,
                        op=mybir.AluOpType.max)

            def phase_D(sg):
                s3g = live[sg]["s3g"]
                prg = live[sg]["prg"]
                ps3 = psX.tile([128, 2, TG], F32, tag="psX")
                nc.tensor.matmul(
                    ps3[:], ones128[0:64, :],
                    bass.AP(tensor=s3g.tensor, offset=s3g[:].offset,
                            ap=[s3g[:].ap[0], [1, 2 * TG]]),
                    start=True, stop=True)
                mean3 = sx.tile([128, TG], F32, tag="mean3")
                nc.vector.tensor_scalar(
                    mean3[:], ps3[:, 0, :], 1.0 / K3, None,
                    mybir.AluOpType.mult)
                nm3 = sx.tile([128, TG], F32, tag="nm3")
                nc.vector.tensor_scalar(
                    nm3[:], mean3[:], -1.0, None, mybir.AluOpType.mult)
                m3sq = sx.tile([128, TG], F32, tag="m3sq")
                nc.vector.tensor_tensor(
                    m3sq[:], mean3[:], mean3[:], mybir.AluOpType.mult)
                var3 = sx.tile([128, TG], F32, tag="var3")
                nc.vector.tensor_scalar(
                    var3[:], m3sq[:], -float(K3) / (K3 - 1), None,
                    mybir.AluOpType.mult)
                ssn = sx.tile([128, TG], F32, tag="ssn")
                nc.vector.tensor_scalar(
                    ssn[:], ps3[:, 1, :], 1.0 / (K3 - 1), None,
                    mybir.AluOpType.mult)
                nc.vector.tensor_tensor(
                    var3[:], var3[:], ssn[:], mybir.AluOpType.add)
                std3 = sx.tile([128, TG], F32, tag="std3")
                nc.scalar.activation(
                    std3[:], var3[:], mybir.ActivationFunctionType.Sqrt,
                    bias=0.0, scale=1.0)
                nc.vector.tensor_scalar(
                    std3[:], std3[:], 1e-5, None, mybir.AluOpType.add)
                sinv3 = sx.tile([128, TG], F32, tag="sinv3")
                nc.vector.reciprocal(sinv3[:], std3[:])

                # pooled = sinv3 * (max(max_m z3, 0) - mean3)
                paug = sx.tile([66, TG], F32, tag="paug")
                nc.vector.memset(paug[64:66, :], 1.0)
                r1 = sx.tile([64, TG], F32, tag="r1")
                nc.vector.tensor_scalar(
                    r1[:], prg[:], 0.0, None, mybir.AluOpType.max)
                r2 = sx.tile([64, TG], F32, tag="r2")
                nc.vector.tensor_tensor(
                    r2[:], r1[:], nm3[0:64, :], mybir.AluOpType.add)
                nc.vector.tensor_tensor(
                    paug[0:64, :], r2[:], sinv3[0:64, :],
                    mybir.AluOpType.mult)

                # h = leaky(W4 @ pooled + b4); out = h @ W5.T + b5
                ph = psX.tile([TG, 32], F32, tag="psX")
                nc.tensor.matmul(ph[:], paug[:, :], mlp_rhs[0:66, :],
                                 start=True, stop=True)
                h = sx.tile([TG, 32], F32, tag="h")
                nc.scalar.activation(
                    h[:], ph[:], mybir.ActivationFunctionType.Prelu,
                    bias=0.0, scale=1.0, alpha=_ALPHA)
                prod = sx.tile([TG, 32], F32, tag="prod")
                nc.vector.tensor_tensor(
                    prod[:], h[:], w5rep[0:TG, :], mybir.AluOpType.mult)
                ov = sx.tile([TG, 1], F32, tag="ov")
                nc.vector.tensor_reduce(
                    ov[:], prod[:], axis=mybir.AxisListType.X,
                    op=mybir.AluOpType.add)
                nc.vector.tensor_scalar(
                    ov[:], ov[:], b5rep[0:TG, :], None,
                    mybir.AluOpType.add)
                nc.sync.dma_start(
                    out=out_dram[sg * TG: (sg + 1) * TG, :], in_=ov[:])

            phase_in(0)
            onesrow = cp.tile([1, 128], F16, tag="onesrow")
            nc.sync.dma_start(out=onesrow[:], in_=ones_in[:])
            ones128 = cp.tile([128, 128], F16, tag="ones128")
            nc.sync.dma_start(out=ones128[:], in_=ones128_in[:])
            brows = cp.tile([3, 256], F16, tag="brows")
            nc.sync.dma_start(out=brows[:], in_=brow_in[:])
            mlp_rhs = cp.tile([66, 32], F32, tag="mlp_rhs")
            nc.sync.dma_start(out=mlp_rhs[:], in_=mlp_in[:])
            w5rep = cp.tile([128, 32], F32, tag="w5rep")
            nc.sync.dma_start(out=w5rep[:], in_=w5_in[:])
            b5rep = cp.tile([128, 1], F32, tag="b5rep")
            nc.sync.dma_start(out=b5rep[:], in_=b5_in[:])
            for i in range(1, NSG + 3):
                if i < NSG:
                    phase_in(i)
                for s in range(SUBS):
                    if 0 <= i - 1 < NSG:
                        phase_A(i - 1, s)
                    if 0 <= i - 2 < NSG:
                        phase_B(i - 2, s)
                    if 0 <= i - 3 < NSG:
                        phase_C(i - 3, s)
                if 0 <= i - 3 < NSG:
                    phase_D(i - 3)
                    del live[i - 3]

    _bass_rust.generate_event_semaphores(nc)
    nc.finalize()
    return nc


_NC_CACHE = {}


def _get_nc(with_bias: bool):
    if with_bias not in _NC_CACHE:
        _NC_CACHE[with_bias] = build_nc(with_bias)
    return _NC_CACHE[with_bias]


def _prep_kmajor_idx(indexes: np.ndarray) -> np.ndarray:
    """indexes [B, 381] -> [B, 384] int32 k-major with dummy col 0 per
    128-block: block k, col m>=1 = idx of triple position m-1, entry k."""
    b = indexes.shape[0]
    tri = indexes.reshape(b, 127, 3)
    karr = np.zeros((b, 3, 128), np.int32)
    karr[:, :, 1:] = tri.transpose(0, 2, 1)
    return karr.reshape(b, 384)


def kernel(trees, W1, b1, W2, b2, W3, b3, W4, b4, W5, b5, indexes):
    trees = np.asarray(trees, dtype=np.float32)
    indexes = np.asarray(indexes).astype(np.int64)
    W1 = np.asarray(W1, dtype=np.float32)
    W2 = np.asarray(W2, dtype=np.float32)
    W3 = np.asarray(W3, dtype=np.float32)
    W4 = np.asarray(W4, dtype=np.float32)
    W5 = np.asarray(W5, dtype=np.float32)
    b1 = np.asarray(b1, dtype=np.float32)
    b2 = np.asarray(b2, dtype=np.float32)
    b3 = np.asarray(b3, dtype=np.float32)
    b4 = np.asarray(b4, dtype=np.float32)
    b5 = np.asarray(b5, dtype=np.float32)

    with_bias = bool(np.any(b1) or np.any(b2) or np.any(b3))
    nc = _get_nc(with_bias)

    # replicated weight prep (fp16)
    w1tT = np.ascontiguousarray(W1.transpose(1, 2, 0)).astype(np.float16)
    w2tT = np.ascontiguousarray(
        W2.reshape(128, 2, 128, 3).transpose(2, 1, 3, 0)).astype(np.float16)
    w3tT = np.ascontiguousarray(W3.transpose(1, 2, 0)).astype(np.float16)
    onesrow = np.ones((1, 128), np.float16)
    ones128 = np.ones((128, 128), np.float16)
    brows = np.zeros((3, 256), np.float16)
    brows[0, :] = b1
    brows[1, :128] = b2
    brows[2, :64] = b3
    mlp_rhs = np.zeros((66, 32), np.float32)
    mlp_rhs[:64] = W4.T
    mlp_rhs[64] = b4 * 0.5
    mlp_rhs[65] = b4 * 0.5
    w5rep = np.tile(W5.reshape(1, 32), (128, 1)).astype(np.float32)
    b5rep = np.full((128, 1), b5[0], np.float32)

    kidx = _prep_kmajor_idx(indexes)                     # [B, 384] int32
    trees16 = trees.astype(np.float16)                   # [B, 128, 128]
    # host L1 gather: G1[b, c, j] = trees16[b, c, kidx[b, j]]
    g1_full = np.take_along_axis(trees16, kidx[:, None, :], axis=2)
    # one-hot S[p, b, j] = (kidx[b, j] == p)
    s_full = (kidx[None, :, :] == np.arange(128, dtype=np.int32)[:, None, None]
              ).astype(np.float16)                       # [128, B, 384]
    # zero the dummy col of each 128-block so gathered col 0 is exactly 0
    # (makes conv-output row 0 equal 0 -> Prelu writes leaky(-mean) there)
    s_full[:, :, 0::128] = 0.0

    # host-precomputed L1 LN mean: z1 sums from the fp16 pre-gather + fp16
    # weights (matches device arithmetic closely; exactness not required
    # since a tiny mean shift cancels through LN2)
    g1s = g1_full.reshape(B, 128, 3, 128)[:, :, :, 1:].astype(
        np.float32).sum(axis=3)                          # [B, 128c, 3k]
    w1s = w1tT.astype(np.float32).sum(axis=2)            # [128c, 3k]
    z1sum = np.einsum('bck,ck->b', g1s, w1s) + 127.0 * float(b1.sum())
    nm1host = (-z1sum / K1).astype(np.float16)           # [B]

    in_maps = []
    for c in range(N_CORES):
        lo, hi = c * BC, (c + 1) * BC
        g1P = np.ascontiguousarray(g1_full[lo:hi].transpose(1, 0, 2))
        sP = np.ascontiguousarray(s_full[:, lo:hi, :])
        in_maps.append({
            "g1D": g1P,
            "sD": sP,
            "nm1D": np.ascontiguousarray(np.tile(nm1host[None, lo:hi], (128, 1))),
            "w1tT": w1tT, "w2tT": w2tT, "w3tT": w3tT,
            "onesrow": onesrow, "ones128": ones128,
            "brows": brows,
            "mlp_rhs": mlp_rhs, "w5rep": w5rep, "b5rep": b5rep,
        })

    global _LAST_IN_MAPS
    _LAST_IN_MAPS = in_maps
    import time as _time
    _t0 = _time.time()
    res = run_bass_kernel_spmd(nc, in_maps, list(range(N_CORES)))
    if _TIME_RUN:
        print(f"  [kernel] device run+transfer: {_time.time() - _t0:.2f}s")
    out = np.concatenate([res.results[c]["out"] for c in range(N_CORES)], axis=0)
    return out.astype(np.float32)


_LAST_IN_MAPS = None
_TIME_RUN = False



# revision 37
# speedup vs baseline: 1.0073x; 1.0073x over previous
"""Trainium2 Bass kernel for nn_BaoCypherNet (tree-conv GNN).

Data-parallel over 8 NeuronCores: each core processes 256 trees.

v6 design — gathers as one-hot matmuls on the PE (the v1 GPSIMD
ap_gather was the 8.8ms bottleneck; SWDGE dma_gather is descriptor-
capped; DMA xbar transpose measured ~26 GB/s — all rejected).

Key structure:
  - Activations live NODE-MAJOR (X^T[node, channel], fp16) so each
    layer chains without transposes:
      gather:  G[c, m] = sum_n X^T[n, c] * S[n, m]   (X^T stationary,
               host-built one-hot S [n, 3*128] streams; S is shared by
               layers 2 and 3 since the indices repeat)
      conv:    Z^T[m, o] = sum_c G[c, m] * W^T[c, o]  (gathered G chunk
               stationary, replicated weights stream)
    Z^T lands node-major in PSUM, so the ACT Prelu(PSUM->SBUF) writes
    the next layer's X^T directly.
  - Layer 1's gather depends only on the inputs, so the HOST
    pre-gathers trees into G1 [c, tree, 384] fp16; it also precomputes
    the L1 LN mean exactly (nm1D, replicated across partitions).
  - S dummy columns (col 0 of each 128-block) are ZEROED on the host,
    so every conv's node-0 output row is exactly 0 pre-mean; the LN
    mean-subtract is fused into the Prelu bias (per-partition AP), so
    node 0 becomes leaky(-mean) with no fixup ops and no K=1
    mean-broadcast matmuls on the PE.
  - LN scale is deferred through layers 1-2 (leaky(s*x) = s*leaky(x));
    only LN3's variance is computed (Square+reduce), batched with the
    whole stats/MLP tail once per 16-tree supergroup.
  - PSUM: two half-bank z1 tiles rotate (next pair's convs only wait
    on the previous pair's Prelus); one shared 4-slot pool (psX) for
    all gather outputs + small stats matmuls.
  - Gather-output PSUM->SBUF copies are split between ACT and DVE to
    balance the two (both ~73% busy, PE ~75%).
  - Supergroups of 16 trees; phases A/B/C/D software-pipelined one
    supergroup apart; input DMA sliced per 4-tree subgroup so compute
    starts early.
"""

import numpy as np

import bass_rust as _bass_rust
import concourse.bass as bass
import concourse.mybir as mybir
from concourse.bass_utils import run_bass_kernel_spmd
from concourse.tile import TileContext

F32 = mybir.dt.float32
F16 = mybir.dt.float16

N_CORES = 8
B = 2048
BC = B // N_CORES   # trees per core (256)
N = 128             # nodes (incl. zero-pad node 0)
TG = 16             # trees per supergroup
TC = 4              # trees per compute sub-group
NSG = BC // TG      # supergroups per core (16)
SUBS = TG // TC     # sub-groups per supergroup (4)
K1 = 256 * 128      # LN element counts per tree
K2 = 128 * 128
K3 = 64 * 128

_ALPHA = 0.01


def _ap(t_ap, extra_dims, offset_delta=0):
    """AP on the same tensor: keep partition dim, replace free dims."""
    return bass.AP(
        tensor=t_ap.tensor,
        offset=t_ap.offset + offset_delta,
        ap=[t_ap.ap[0]] + list(extra_dims),
    )


def build_nc(with_bias: bool):
    nc = bass.Bass()

    g1_in = nc.dram_tensor("g1D", [128, BC, 384], F16, kind="ExternalInput")
    s_in = nc.dram_tensor("sD", [128, BC, 384], F16, kind="ExternalInput")
    nm1_in = nc.dram_tensor("nm1D", [128, BC], F16, kind="ExternalInput")
    w1_in = nc.dram_tensor("w1tT", [128, 3, 256], F16, kind="ExternalInput")
    w2_in = nc.dram_tensor("w2tT", [128, 2, 3, 128], F16, kind="ExternalInput")
    w3_in = nc.dram_tensor("w3tT", [128, 3, 64], F16, kind="ExternalInput")
    ones_in = nc.dram_tensor("onesrow", [1, 128], F16, kind="ExternalInput")
    ones128_in = nc.dram_tensor("ones128", [128, 128], F16, kind="ExternalInput")
    brow_in = nc.dram_tensor("brows", [3, 256], F16, kind="ExternalInput")
    mlp_in = nc.dram_tensor("mlp_rhs", [66, 32], F32, kind="ExternalInput")
    w5_in = nc.dram_tensor("w5rep", [128, 32], F32, kind="ExternalInput")
    b5_in = nc.dram_tensor("b5rep", [128, 1], F32, kind="ExternalInput")
    out_dram = nc.dram_tensor("out", [BC, 1], F32, kind="ExternalOutput")

    with TileContext(nc) as tc:
        with (
            tc.tile_pool(name="const", bufs=1) as cp,
            tc.tile_pool(name="sx", bufs=2) as sx,
            tc.tile_pool(name="gp", bufs=2) as gp,
            tc.tile_pool(name="psZ1", bufs=2, space="PSUM") as psZ1,
            tc.tile_pool(name="psB", bufs=2, space="PSUM") as psB,
            tc.tile_pool(name="psX", bufs=4, space="PSUM") as psX,
        ):
            # ---- constants ----
            w1tT = cp.tile([128, 3, 256], F16, tag="w1tT")
            nc.sync.dma_start(out=w1tT[:], in_=w1_in[:])
            w2tT = cp.tile([128, 2, 3, 128], F16, tag="w2tT")
            nc.sync.dma_start(out=w2tT[:], in_=w2_in[:])
            w3tT = cp.tile([128, 3, 64], F16, tag="w3tT")
            nc.sync.dma_start(out=w3tT[:], in_=w3_in[:])
            nm1sb = cp.tile([128, BC], F16, tag="nm1sb")
            nc.sync.dma_start(out=nm1sb[:], in_=nm1_in[:])

            live = {}

            def phase_in(i):
                # per-subgroup DMA slices so compute on the first trees can
                # start before the whole supergroup has landed
                g1 = gp.tile([128, TG, 384], F16, tag="g1")
                st = gp.tile([128, TG, 384], F16, tag="st", bufs=4)
                for s in range(SUBS):
                    lo, hi = s * TC, (s + 1) * TC
                    nc.sync.dma_start(
                        out=g1[:, lo:hi, :],
                        in_=g1_in[:, i * TG + lo:i * TG + hi, :])
                    nc.sync.dma_start(
                        out=st[:, lo:hi, :],
                        in_=s_in[:, i * TG + lo:i * TG + hi, :])
                live[i] = {"g1": g1, "s": st}

            def phase_A(sg, s):
                g1 = live[sg]["g1"]
                if s == 0:
                    live[sg]["x2t"] = sx.tile([128, TG, 256], F16, tag="x2t", name="x2t")
                x2t = live[sg]["x2t"]
                if True:
                    t0 = s * TC
                    # two half-tiles (1 PSUM bank each, 2-buf rotation) so
                    # the next pair's convs only wait on this pair's Prelus
                    for h in range(2):
                        z1 = psZ1.tile([128, 2, 256], F32, tag="z1")
                        first = True
                        if with_bias:
                            nc.tensor.matmul(
                                z1[:], onesrow[:],
                                _ap(brows[0:1, :], [[0, 2], [1, 256]]),
                                start=True, stop=False, skip_group_check=True)
                            first = False
                        for u in range(2):
                            t = 2 * h + u
                            for k in range(3):
                                nc.tensor.matmul(
                                    z1[:, u, :],
                                    g1[:, t0 + t, k * 128:(k + 1) * 128],
                                    w1tT[:, k, :],
                                    start=(first and k == 0 and u == 0),
                                    stop=(k == 2 and u == 1),
                                    skip_group_check=True)
                        # mean subtract fused into the Prelu bias (host
                        # precomputed -mean, replicated across partitions).
                        # z1 row 0 = 0 (dummy col) -> leaky(-mean).
                        for u in range(2):
                            t = 2 * h + u
                            nc.scalar.activation(
                                x2t[:, t0 + t, :], z1[:, u, :],
                                mybir.ActivationFunctionType.Prelu,
                                bias=nm1sb[:, sg * TG + t0 + t:
                                           sg * TG + t0 + t + 1],
                                scale=1.0, alpha=_ALPHA)

            def phase_B(sg, s):
                st = live[sg]["s"]
                x2t = live[sg]["x2t"]
                if s == 0:
                    live[sg]["x3t"] = sx.tile([128, TG, 128], F16, tag="x3t", name="x3t")
                x3t = live[sg]["x3t"]
                if True:
                    t0 = s * TC
                    # gather via one-hot matmul: G2[c,m] = X2^T . S
                    z2 = psB.tile([128, TC, 128], F32, tag="zB")
                    first = True
                    if with_bias:
                        nc.tensor.matmul(
                            z2[:], onesrow[:],
                            _ap(brows[1:2, :], [[0, TC], [1, 128]]),
                            start=True, stop=False, skip_group_check=True)
                        first = False
                    # per tree: gather both c-halves, copy, then its convs
                    # immediately -- PE fills copy latency with conv work
                    for t in range(TC):
                        gsb = sx.tile([128, 2, 384], F16, tag=f"g2sb{t % 2}")
                        for j in range(2):
                            gps = psX.tile([128, 384], F32, tag="psX")
                            nc.tensor.matmul(
                                gps[:], x2t[:, t0 + t, j * 128:(j + 1) * 128],
                                st[:, t0 + t, :], start=True, stop=True)
                            with nc.allow_low_precision(reason="fp16 acts"):
                                if j == 1:
                                    nc.vector.tensor_copy(
                                        gsb[:, j, :], gps[:])
                                else:
                                    nc.scalar.activation(
                                        gsb[:, j, :], gps[:],
                                        mybir.ActivationFunctionType.Copy,
                                        bias=0.0, scale=1.0)
                        for k in range(3):
                            for j in range(2):
                                nc.tensor.matmul(
                                    z2[:, t, :],
                                    gsb[:, j, k * 128:(k + 1) * 128],
                                    w2tT[:, j, k, :],
                                    start=(first and k == 0 and j == 0
                                           and t == 0),
                                    stop=(t == TC - 1 and k == 2 and j == 1),
                                    skip_group_check=True)
                    s2 = sx.tile([128, TC], F16, tag="s2")
                    with nc.allow_low_precision(reason="LN mean in fp16 ok"):
                        nc.vector.tensor_reduce(
                            s2[:], z2[:], axis=mybir.AxisListType.X,
                            op=mybir.AluOpType.add)
                    ps2 = psX.tile([128, TC], F32, tag="psX")
                    nc.tensor.matmul(ps2[:], ones128[:], s2[:],
                                     start=True, stop=True)
                    nm2 = sx.tile([128, TC], F16, tag="nm2")
                    with nc.allow_low_precision(reason="LN mean in fp16 ok"):
                        nc.vector.tensor_scalar(
                            nm2[:], ps2[:], -1.0 / K2, None, mybir.AluOpType.mult)
                    # z2 row 0 = 0 by construction (S dummy cols zeroed
                    # on host); mean subtract fused into the Prelu bias.
                    for t in range(TC):
                        nc.scalar.activation(
                            x3t[:, t0 + t, :], z2[:, t, :],
                            mybir.ActivationFunctionType.Prelu,
                            bias=nm2[:, t:t + 1], scale=1.0, alpha=_ALPHA)

            def phase_C(sg, s):
                st = live[sg]["s"]
                x3t = live[sg]["x3t"]
                if True:
                    t0 = s * TC
                    g3sb = sx.tile([128, TC, 384], F16, tag="g3sb")
                    for t in range(TC):
                        gps = psX.tile([128, 384], F32, tag="psX")
                        nc.tensor.matmul(
                            gps[:], x3t[:, t0 + t, :], st[:, t0 + t, :],
                            start=True, stop=True)
                        with nc.allow_low_precision(reason="fp16 acts"):
                            if t % 2 == 1:
                                nc.vector.tensor_copy(
                                    g3sb[:, t, :], gps[:])
                            else:
                                nc.scalar.activation(
                                    g3sb[:, t, :], gps[:],
                                    mybir.ActivationFunctionType.Copy,
                                    bias=0.0, scale=1.0)
                    zb = psB.tile([128, TC, 128], F32, tag="zB")
                    z3 = zb[0:64, :, :]
                    for h in range(2):
                        for k in range(3):
                            rhs = bass.AP(
                                tensor=g3sb.tensor,
                                offset=(g3sb[:].offset + 2 * h * 384
                                        + k * 128),
                                ap=[g3sb[:].ap[0], [384, 2], [1, 128]])
                            nc.tensor.matmul(
                                zb[0:64, 2 * h:2 * h + 2, :],
                                w3tT[:, k, :], rhs,
                                start=(k == 0),
                                stop=(k == 2 and not with_bias),
                                skip_group_check=True)
                    if with_bias:
                        nc.tensor.matmul(
                            z3, brows[2:3, 0:64],
                            _ap(onesrow[:], [[0, TC], [0, 128]]),
                            start=False, stop=True, skip_group_check=True)
                    z3v = zb[0:64, :, 1:128]
                    # LN3 raw stats accumulate into per-supergroup tiles;
                    # the scalar pipeline runs once per supergroup (phase_D)
                    if s == 0:
                        live[sg]["s3g"] = sx.tile([64, 2, TG], F16,
                                                  tag="s3g", name="s3g")
                        live[sg]["prg"] = sx.tile([64, TG], F32,
                                                  tag="prg", name="prg")
                    s3g = live[sg]["s3g"]
                    prg = live[sg]["prg"]
                    sq = sx.tile([64, TC, 127], F16, tag="sq")
                    with nc.allow_low_precision(reason="stats fp16 ok"):
                        nc.vector.tensor_reduce(
                            s3g[:, 0, t0:t0 + TC], z3v,
                            axis=mybir.AxisListType.X, op=mybir.AluOpType.add)
                        nc.scalar.activation(
                            sq[:], z3v, mybir.ActivationFunctionType.Square,
                            bias=0.0, scale=1.0)
                        nc.vector.tensor_reduce(
                            s3g[:, 1, t0:t0 + TC], sq[:],
                            axis=mybir.AxisListType.X, op=mybir.AluOpType.add)
                    nc.vector.tensor_reduce(
                        prg[:, t0:t0 + TC], z3v, axis=mybir.AxisListType.X,
                        op=mybir.AluOpType.max)

            def phase_D(sg):
                s3g = live[sg]["s3g"]
                prg = live[sg]["prg"]
                ps3 = psX.tile([128, 2, TG], F32, tag="psX")
                nc.tensor.matmul(
                    ps3[:], ones128[0:64, :],
                    bass.AP(tensor=s3g.tensor, offset=s3g[:].offset,
                            ap=[s3g[:].ap[0], [1, 2 * TG]]),
                    start=True, stop=True)
                mean3 = sx.tile([128, TG], F32, tag="mean3")
                nc.vector.tensor_scalar(
                    mean3[:], ps3[:, 0, :], 1.0 / K3, None,
                    mybir.AluOpType.mult)
                nm3 = sx.tile([128, TG], F32, tag="nm3")
                nc.vector.tensor_scalar(
                    nm3[:], mean3[:], -1.0, None, mybir.AluOpType.mult)
                m3sq = sx.tile([128, TG], F32, tag="m3sq")
                nc.vector.tensor_tensor(
                    m3sq[:], mean3[:], mean3[:], mybir.AluOpType.mult)
                var3 = sx.tile([128, TG], F32, tag="var3")
                nc.vector.tensor_scalar(
                    var3[:], m3sq[:], -float(K3) / (K3 - 1), None,
                    mybir.AluOpType.mult)
                ssn = sx.tile([128, TG], F32, tag="ssn")
                nc.vector.tensor_scalar(
                    ssn[:], ps3[:, 1, :], 1.0 / (K3 - 1), None,
                    mybir.AluOpType.mult)
                nc.vector.tensor_tensor(
                    var3[:], var3[:], ssn[:], mybir.AluOpType.add)
                std3 = sx.tile([128, TG], F32, tag="std3")
                nc.scalar.activation(
                    std3[:], var3[:], mybir.ActivationFunctionType.Sqrt,
                    bias=0.0, scale=1.0)
                nc.vector.tensor_scalar(
                    std3[:], std3[:], 1e-5, None, mybir.AluOpType.add)
                sinv3 = sx.tile([128, TG], F32, tag="sinv3")
                nc.vector.reciprocal(sinv3[:], std3[:])

                # pooled = sinv3 * (max(max_m z3, 0) - mean3)
                paug = sx.tile([66, TG], F32, tag="paug")
                nc.vector.memset(paug[64:66, :], 1.0)
                r1 = sx.tile([64, TG], F32, tag="r1")
                nc.vector.tensor_scalar(
                    r1[:], prg[:], 0.0, None, mybir.AluOpType.max)
                r2 = sx.tile([64, TG], F32, tag="r2")
                nc.vector.tensor_tensor(
                    r2[:], r1[:], nm3[0:64, :], mybir.AluOpType.add)
                nc.vector.tensor_tensor(
                    paug[0:64, :], r2[:], sinv3[0:64, :],
                    mybir.AluOpType.mult)

                # h = leaky(W4 @ pooled + b4); out = h @ W5.T + b5
                ph = psX.tile([TG, 32], F32, tag="psX")
                nc.tensor.matmul(ph[:], paug[:, :], mlp_rhs[0:66, :],
                                 start=True, stop=True)
                h = sx.tile([TG, 32], F32, tag="h")
                nc.scalar.activation(
                    h[:], ph[:], mybir.ActivationFunctionType.Prelu,
                    bias=0.0, scale=1.0, alpha=_ALPHA)
                prod = sx.tile([TG, 32], F32, tag="prod")
                nc.vector.tensor_tensor(
                    prod[:], h[:], w5rep[0:TG, :], mybir.AluOpType.mult)
                ov = sx.tile([TG, 1], F32, tag="ov")
                nc.vector.tensor_reduce(
                    ov[:], prod[:], axis=mybir.AxisListType.X,
                    op=mybir.AluOpType.add)
                nc.vector.tensor_scalar(
                    ov[:], ov[:], b5rep[0:TG, :], None,
                    mybir.AluOpType.add)
                nc.sync.dma_start(
                    out=out_dram[sg * TG: (sg + 1) * TG, :], in_=ov[:])

            phase_in(0)
            onesrow = cp.tile([1, 128], F16, tag="onesrow")
            nc.sync.dma_start(out=onesrow[:], in_=ones_in[:])
            ones128 = cp.tile([128, 128], F16, tag="ones128")
            nc.sync.dma_start(out=ones128[:], in_=ones128_in[:])
            brows = cp.tile([3, 256], F16, tag="brows")
            nc.sync.dma_start(out=brows[:], in_=brow_in[:])
            mlp_rhs = cp.tile([66, 32], F32, tag="mlp_rhs")
            nc.sync.dma_start(out=mlp_rhs[:], in_=mlp_in[:])
            w5rep = cp.tile([128, 32], F32, tag="w5rep")
            nc.sync.dma_start(out=w5rep[:], in_=w5_in[:])
            b5rep = cp.tile([128, 1], F32, tag="b5rep")
            nc.sync.dma_start(out=b5rep[:], in_=b5_in[:])
            for i in range(1, NSG + 3):
                if i < NSG:
                    phase_in(i)
                for s in range(SUBS):
                    if 0 <= i - 1 < NSG:
                        phase_A(i - 1, s)
                    if 0 <= i - 2 < NSG:
                        phase_B(i - 2, s)
                    if 0 <= i - 3 < NSG:
                        phase_C(i - 3, s)
                if 0 <= i - 3 < NSG:
                    phase_D(i - 3)
                    del live[i - 3]

    _bass_rust.generate_event_semaphores(nc)
    nc.finalize()
    return nc


_NC_CACHE = {}


def _get_nc(with_bias: bool):
    if with_bias not in _NC_CACHE:
        _NC_CACHE[with_bias] = build_nc(with_bias)
    return _NC_CACHE[with_bias]


def _prep_kmajor_idx(indexes: np.ndarray) -> np.ndarray:
    """indexes [B, 381] -> [B, 384] int32 k-major with dummy col 0 per
    128-block: block k, col m>=1 = idx of triple position m-1, entry k."""
    b = indexes.shape[0]
    tri = indexes.reshape(b, 127, 3)
    karr = np.zeros((b, 3, 128), np.int32)
    karr[:, :, 1:] = tri.transpose(0, 2, 1)
    return karr.reshape(b, 384)


def kernel(trees, W1, b1, W2, b2, W3, b3, W4, b4, W5, b5, indexes):
    trees = np.asarray(trees, dtype=np.float32)
    indexes = np.asarray(indexes).astype(np.int64)
    W1 = np.asarray(W1, dtype=np.float32)
    W2 = np.asarray(W2, dtype=np.float32)
    W3 = np.asarray(W3, dtype=np.float32)
    W4 = np.asarray(W4, dtype=np.float32)
    W5 = np.asarray(W5, dtype=np.float32)
    b1 = np.asarray(b1, dtype=np.float32)
    b2 = np.asarray(b2, dtype=np.float32)
    b3 = np.asarray(b3, dtype=np.float32)
    b4 = np.asarray(b4, dtype=np.float32)
    b5 = np.asarray(b5, dtype=np.float32)

    with_bias = bool(np.any(b1) or np.any(b2) or np.any(b3))
    nc = _get_nc(with_bias)

    # replicated weight prep (fp16)
    w1tT = np.ascontiguousarray(W1.transpose(1, 2, 0)).astype(np.float16)
    w2tT = np.ascontiguousarray(
        W2.reshape(128, 2, 128, 3).transpose(2, 1, 3, 0)).astype(np.float16)
    w3tT = np.ascontiguousarray(W3.transpose(1, 2, 0)).astype(np.float16)
    onesrow = np.ones((1, 128), np.float16)
    ones128 = np.ones((128, 128), np.float16)
    brows = np.zeros((3, 256), np.float16)
    brows[0, :] = b1
    brows[1, :128] = b2
    brows[2, :64] = b3
    mlp_rhs = np.zeros((66, 32), np.float32)
    mlp_rhs[:64] = W4.T
    mlp_rhs[64] = b4 * 0.5
    mlp_rhs[65] = b4 * 0.5
    w5rep = np.tile(W5.reshape(1, 32), (128, 1)).astype(np.float32)
    b5rep = np.full((128, 1), b5[0], np.float32)

    kidx = _prep_kmajor_idx(indexes)                     # [B, 384] int32
    trees16 = trees.astype(np.float16)                   # [B, 128, 128]
    # host L1 gather: G1[b, c, j] = trees16[b, c, kidx[b, j]]
    g1_full = np.take_along_axis(trees16, kidx[:, None, :], axis=2)
    # one-hot S[p, b, j] = (kidx[b, j] == p)
    s_full = (kidx[None, :, :] == np.arange(128, dtype=np.int32)[:, None, None]
              ).astype(np.float16)                       # [128, B, 384]
    # zero the dummy col of each 128-block so gathered col 0 is exactly 0
    # (makes conv-output row 0 equal 0 -> Prelu writes leaky(-mean) there)
    s_full[:, :, 0::128] = 0.0

    # host-precomputed L1 LN mean: z1 sums from the fp16 pre-gather + fp16
    # weights (matches device arithmetic closely; exactness not required
    # since a tiny mean shift cancels through LN2)
    g1s = g1_full.reshape(B, 128, 3, 128)[:, :, :, 1:].astype(
        np.float32).sum(axis=3)                          # [B, 128c, 3k]
    w1s = w1tT.astype(np.float32).sum(axis=2)            # [128c, 3k]
    z1sum = np.einsum('bck,ck->b', g1s, w1s) + 127.0 * float(b1.sum())
    nm1host = (-z1sum / K1).astype(np.float16)           # [B]

    in_maps = []
    for c in range(N_CORES):
        lo, hi = c * BC, (c + 1) * BC
        g1P = np.ascontiguousarray(g1_full[lo:hi].transpose(1, 0, 2))
        sP = np.ascontiguousarray(s_full[:, lo:hi, :])
        in_maps.append({
            "g1D": g1P,
            "sD": sP,
            "nm1D": np.ascontiguousarray(np.tile(nm1host[None, lo:hi], (128, 1))),
            "w1tT": w1tT, "w2tT": w2tT, "w3tT": w3tT,
            "onesrow": onesrow, "ones128": ones128,
            "brows": brows,
            "mlp_rhs": mlp_rhs, "w5rep": w5rep, "b5rep": b5rep,
        })

    global _LAST_IN_MAPS
    _LAST_IN_MAPS = in_maps
    import time as _time
    _t0 = _time.time()
    res = run_bass_kernel_spmd(nc, in_maps, list(range(N_CORES)))
    if _TIME_RUN:
        print(f"  [kernel] device run+transfer: {_time.time() - _t0:.2f}s")
    out = np.concatenate([res.results[c]["out"] for c in range(N_CORES)], axis=0)
    return out.astype(np.float32)


_LAST_IN_MAPS = None
_TIME_RUN = False

